# revision 1
# baseline (speedup 1.0000x reference)
"""Trainium2 Bass kernel for the 2D-patch LSTM (nn_Lstm2D).

Math (reference): row-major scan over 32x32 grid of 8x8 patches of a
(64,3,256,256) image. Per step t: gates = [x_t, h_{t-32}] @ W_ih.T +
h_{t-1} @ W_hh.T + b; standard LSTM cell update (i,f,g,o torch order).
Strictly sequential chain of T=1024 steps (h,c carry across row
boundaries), so the kernel runs the full scan per core and shards the
batch: 8 cores x 8 samples, weights replicated.

Device-side formulation:
  - all matmuls in bf16 (fp32 PSUM accumulate); weight rows permuted so
    PSUM m-tile (4k+j) holds gate j of NC-chunk k, j order [i,f,o,g]
  - i/f/o weight rows pre-scaled by 0.5 and state stored as 2h/2c so the
    whole cell update needs only tanh():  sigmoid(x) = (tanh(x/2)+1)/2
  - per 16-step group, x-projection (W_x, + bias via a ones-row) and
    lookback term (W_v @ h_{t-32}) are batched matmuls (N=128)
    pre-accumulated into that group's PSUM gates buffer, interleaved as
    background PE work during the previous group's steps
  - per step only W_hh @ h_{t-1} (64 bf16 matmuls, N=8) + 2 scalar-engine
    tanh ACTs + 5 vector scalar_tensor_tensor ops are on the chain
"""

import numpy as np
import ml_dtypes

import concourse.bass as bass
import concourse.bacc as bacc
import concourse.tile as tile
from concourse import mybir
from concourse.bass import ds
from concourse.bass_utils import run_bass_kernel_spmd

BF16 = mybir.dt.bfloat16
F32 = mybir.dt.float32
AF = mybir.ActivationFunctionType
OP = mybir.AluOpType

NCORES = 8
B, C, H, W = 64, 3, 256, 256
P = 8
NC = 512
F = C * P * P          # 192
G4 = 4 * NC            # 2048
SY = SX = 32
T = SY * SX            # 1024
B1 = B // NCORES       # 8 batch per core
MT = 16                # gate m-tiles of 128
KC = 4                 # NC contraction chunks of 128
SPG = 16               # steps per group (PSUM buffer granularity)
SPB = 32               # steps per loop body (= ring period)
NBODY = T // SPB       # 32

_COMPILED: dict = {}


def _build(nbody: int, repeats: int):
    nc = bacc.Bacc("TRN2", target_bir_lowering=False, debug=False,
                   num_devices=NCORES)
    t_total = nbody * SPB
    xq_d = nc.dram_tensor("xq", [128, 2, t_total + SPB, B1], BF16,
                          kind="ExternalInput").ap()
    whh_d = nc.dram_tensor("whhT", [128, KC * G4], BF16,
                           kind="ExternalInput").ap()
    wv_d = nc.dram_tensor("wvT", [128, KC * G4], BF16,
                          kind="ExternalInput").ap()
    wx_d = nc.dram_tensor("wxT", [128, 2 * G4], BF16,
                          kind="ExternalInput").ap()
    ho_d = nc.dram_tensor("ho", [128, t_total, KC, B1], F32,
                          kind="ExternalOutput").ap()

    with tile.TileContext(nc) as tc:
        with (
            tc.tile_pool(name="persist", bufs=1) as pp,
            tc.tile_pool(name="ew", bufs=3) as ew,
            tc.tile_pool(name="psum", bufs=1, space="PSUM") as psp,
        ):
            w_hh = pp.tile([128, KC * G4], BF16, tag="w_hh")
            w_v = pp.tile([128, KC * G4], BF16, tag="w_v")
            w_x = pp.tile([128, 2 * G4], BF16, tag="w_x")
            ring = pp.tile([128, SPB, KC, B1], BF16, tag="ring")
            c2 = [pp.tile([128, KC, B1], F32, tag=f"c2_{i}", name=f"c2_{i}") for i in (0, 1)]
            xq_t = [pp.tile([128, 2, SPG, B1], BF16, tag=f"xq_{i}", name=f"xq_{i}")
                    for i in (0, 1)]  # [even-group, odd-group]
            hout = [pp.tile([128, SPG, KC, B1], F32, tag=f"hout_{i}",
                            name=f"hout_{i}") for i in (0, 1)]
            gates = [psp.tile([128, MT, SPG, B1], F32, tag=f"g_{i}", name=f"g_{i}")
                     for i in (0, 1)]  # group parity E/O

            nc.sync.dma_start(w_hh[:], whh_d)
            nc.sync.dma_start(w_v[:], wv_d)
            nc.sync.dma_start(w_x[:], wx_d)
            nc.vector.memset(ring[:], 0.0)
            nc.vector.memset(c2[0][:], 0.0)
            nc.vector.memset(c2[1][:], 0.0)

            def emit_bg(gt, xq, s0):
                """Background matmuls pre-accumulating one group's gates:
                W_v @ h2[t-32] (ring slots s0..s0+15, no DMA dependency,
                emitted first and carrying the bank-clearing start=True)
                then x-projection (+bias row) from the xq tile."""
                ops = []
                for m in range(MT):
                    for k in range(KC):
                        def op(m=m, k=k):
                            nc.tensor.matmul(
                                gt[:, m, :, :],
                                w_v[:, k * G4 + m * 128:k * G4 + (m + 1) * 128],
                                ring[:, s0:s0 + SPG, k, :],
                                start=(k == 0 and m % 4 == 0), stop=False,
                                skip_group_check=True)
                        ops.append(op)
                for m in range(MT):
                    for kc in range(2):
                        def op(m=m, kc=kc):
                            nc.tensor.matmul(
                                gt[:, m, :, :],
                                w_x[:, kc * G4 + m * 128:kc * G4 + (m + 1) * 128],
                                xq[:, kc, :, :],
                                start=False, stop=False, skip_group_check=True)
                        ops.append(op)
                return ops

            def emit_step(lt, bg_ops):
                """One LSTM step lt (0..31) within the body."""
                gl, lt_g = lt // SPG, lt % SPG
                gt = gates[gl]
                prev = (lt - 1) % SPB
                last_step = lt_g == SPG - 1
                for m in range(MT):
                    for k in range(KC):
                        # stop closes the whole PSUM bank's accumulation
                        # group: only on the bank's final matmul (last step
                        # of the group, last m-tile in the bank, last k)
                        nc.tensor.matmul(
                            gt[:, m, lt_g, :],
                            w_hh[:, k * G4 + m * 128:k * G4 + (m + 1) * 128],
                            ring[:, prev, k, :],
                            start=False,
                            stop=(last_step and k == KC - 1 and m % 4 == 3),
                            skip_group_check=True)
                for op in bg_ops:
                    op()
                # elementwise: t_all = tanh(gates); j-order [i,f,o,g]
                t_all = ew.tile([128, MT, B1], F32, tag="t_all", name="t_all")
                nc.scalar.activation(t_all[:], gt[:, :, lt_g, :], AF.Tanh)
                t4 = t_all[:].rearrange("p (k j) b -> p k j b", j=4)
                b2 = ew.tile([128, KC, B1], F32, tag="b2", name="b2")
                a2 = ew.tile([128, KC, B1], F32, tag="a2", name="a2")
                tch = ew.tile([128, KC, B1], F32, tag="tch", name="tch")
                # b2 = (t_i+1)*t_g = 2 si * tanh(g)
                nc.vector.scalar_tensor_tensor(
                    b2[:], t4[:, :, 0, :], 1.0, t4[:, :, 3, :], OP.add, OP.mult)
                # a2 = (t_f+1)*c2_old = 4 sf * c_old
                nc.vector.scalar_tensor_tensor(
                    a2[:], t4[:, :, 1, :], 1.0, c2[1 - lt % 2][:], OP.add, OP.mult)
                # c2_new = a2/2 + b2 = 2 c_new
                nc.vector.scalar_tensor_tensor(
                    c2[lt % 2][:], a2[:], 0.5, b2[:], OP.mult, OP.add)
                # tch = tanh(c_new)
                nc.scalar.activation(tch[:], c2[lt % 2][:], AF.Tanh, scale=0.5)
                # h2 = (t_o+1)*tch = 2 h  -> ring (bf16, feeds matmuls)
                nc.vector.scalar_tensor_tensor(
                    ring[:, lt, :, :], t4[:, :, 2, :], 1.0, tch[:],
                    OP.add, OP.mult)
                # fp32 copy for output
                nc.vector.scalar_tensor_tensor(
                    hout[gl][:, lt_g, :, :], t4[:, :, 2, :], 1.0, tch[:],
                    OP.add, OP.mult)

            # prologue: group 0 inputs + gates
            nc.sync.dma_start(xq_t[0][:], xq_d[:, :, 0:SPG, :])
            for op in emit_bg(gates[0], xq_t[0], 0):
                op()

            def body(i):
                # xq for group 2i+1 (used by bg during gl=0)
                nc.sync.dma_start(xq_t[1][:],
                                  xq_d[:, :, ds(i * SPB + SPG, SPG), :])
                bg = emit_bg(gates[1], xq_t[1], SPG)  # group 2i+1
                nper = (len(bg) + SPG - 1) // SPG
                for lt in range(SPG):
                    emit_step(lt, bg[lt * nper:(lt + 1) * nper])
                # group 0-15 output: DMA'd here so the tile has a full
                # group of slack before body i+1 rewrites it
                nc.sync.dma_start(ho_d[:, ds(i * SPB, SPG), :, :],
                                  hout[0][:])
                # xq for group 2i+2 (used by bg during gl=1)
                nc.sync.dma_start(xq_t[0][:],
                                  xq_d[:, :, ds(i * SPB + SPB, SPG), :])
                bg = emit_bg(gates[0], xq_t[0], 0)    # group 2i+2
                for lt in range(SPG, SPB):
                    emit_step(lt, bg[(lt - SPG) * nper:(lt - SPG + 1) * nper])
                nc.sync.dma_start(ho_d[:, ds(i * SPB + SPG, SPG), :, :],
                                  hout[1][:])

            if repeats == 1:
                with tc.For_i(0, nbody, 1,
                              hint_engines=(mybir.EngineType.PE,)) as i:
                    body(i)
            else:
                with tc.For_i(0, repeats, 1) as _r:
                    with tc.For_i(0, nbody, 1,
                                  hint_engines=(mybir.EngineType.PE,)) as i:
                        body(i)

    nc.compile()
    return nc


def _get(nbody: int, repeats: int):
    key = (nbody, repeats)
    if key not in _COMPILED:
        _COMPILED[key] = _build(nbody, repeats)
    return _COMPILED[key]


def _perm_idx():
    """Permuted gate-row order: m-tile (4k+j) = gate j of NC-chunk k,
    j order [i,f,o,g]; torch gate blocks i=0,f=1,g=2,o=3."""
    gid = [0, 1, 3, 2]
    idx = np.empty(G4, np.int64)
    rs = np.empty(G4, np.float32)
    for k in range(KC):
        for j in range(4):
            m = 4 * k + j
            idx[m * 128:(m + 1) * 128] = 512 * gid[j] + 128 * k + np.arange(128)
            rs[m * 128:(m + 1) * 128] = 0.5 if j < 3 else 1.0
    return idx, rs


def _lhsT_pack(wp: np.ndarray) -> np.ndarray:
    """[G4, 512] permuted+scaled weight -> [128, 4*G4] bf16 lhsT tiles:
    out[p, k*G4 + m*128 + c] = wp[m*128+c, 128k+p]."""
    a = wp.reshape(MT, 128, KC, 128).transpose(3, 2, 0, 1).reshape(128, KC * G4)
    return np.ascontiguousarray(a.astype(ml_dtypes.bfloat16))


def _prep_weights(W_ih, W_hh, b_ih, b_hh):
    idx, rs = _perm_idx()
    bias = (np.asarray(b_ih, np.float32) + np.asarray(b_hh, np.float32))[idx] * rs
    Wih_p = np.asarray(W_ih, np.float32)[idx] * rs[:, None]
    Whh_p = np.asarray(W_hh, np.float32)[idx] * rs[:, None] * 0.5
    Wv_p = Wih_p[:, F:] * 0.5
    Wx_p = Wih_p[:, :F]
    whhT = _lhsT_pack(Whh_p)
    wvT = _lhsT_pack(Wv_p)
    wxT = np.zeros((128, 2 * G4), np.float32)
    # chunk 0: features 0..127 ; chunk 1: features 128..191 + bias row 64
    wxT[:, :G4] = Wx_p.reshape(MT, 128, F)[:, :, :128].transpose(2, 0, 1).reshape(128, G4)
    wxT[:64, G4:] = Wx_p.reshape(MT, 128, F)[:, :, 128:].transpose(2, 0, 1).reshape(64, G4)
    wxT[64, G4:] = bias
    return whhT, wvT, np.ascontiguousarray(wxT.astype(ml_dtypes.bfloat16))


def _prep_xq(x_core: np.ndarray, t_total: int) -> np.ndarray:
    """x_core (B1,C,H,W) -> [128, 2, t_total+SPB, B1] bf16 with ones row."""
    xp = (x_core.reshape(B1, C, SY, P, SX, P)
          .transpose(2, 4, 0, 1, 3, 5).reshape(T, B1, F))
    xpT = xp.transpose(2, 0, 1)  # [F, T, B1]
    xq = np.zeros((128, 2, t_total + SPB, B1), np.float32)
    tt = min(T, t_total)
    xq[:, 0, :tt, :] = xpT[:128, :tt]
    xq[:64, 1, :tt, :] = xpT[128:, :tt]
    xq[64, 1, :tt, :] = 1.0
    return np.ascontiguousarray(xq.astype(ml_dtypes.bfloat16))


def _in_maps(x, W_ih, W_hh, b_ih, b_hh, t_total=T):
    whhT, wvT, wxT = _prep_weights(W_ih, W_hh, b_ih, b_hh)
    x = np.asarray(x, np.float32)
    maps = []
    for j in range(NCORES):
        maps.append({
            "xq": _prep_xq(x[j * B1:(j + 1) * B1], t_total),
            "whhT": whhT, "wvT": wvT, "wxT": wxT,
        })
    return maps


def _assemble(results, t_total=T):
    """results[j]["ho"] [128, t_total, KC, B1] (= 2h) -> (B, NC, SY, SX).

    Matches the reference's to_image exactly: (B, T, NC) row-major data
    reinterpreted as (B, NC, sy, sx) -- T and NC deliberately interleave."""
    out = np.empty((B, t_total, NC), np.float32)
    for j in range(NCORES):
        ho = results[j]["ho"]  # [128(p), t, 4(k), 8(b)] ; nc = 128k+p
        out[j * B1:(j + 1) * B1] = 0.5 * ho.transpose(3, 1, 2, 0).reshape(
            B1, t_total, NC)
    return out.reshape(B, NC, t_total // SX, SX)


def kernel(x, W_ih, W_hh, b_ih, b_hh):
    nc = _get(NBODY, 1)
    maps = _in_maps(x, W_ih, W_hh, b_ih, b_hh)
    res = run_bass_kernel_spmd(nc, maps, core_ids=list(range(NCORES)))
    return _assemble(res.results)



# revision 9
# speedup vs baseline: 65.2649x; 65.2649x over previous
"""Trainium2 Bass kernel for the 2D-patch LSTM (nn_Lstm2D).

Math (reference): row-major scan over 32x32 grid of 8x8 patches of a
(64,3,256,256) image. Per step t: gates = [x_t, h_{t-32}] @ W_ih.T +
h_{t-1} @ W_hh.T + b; standard LSTM cell update (i,f,g,o torch order).
Strictly sequential chain of T=1024 steps (h,c carry across row
boundaries), so the kernel runs the full scan per core and shards the
batch: 8 cores x 8 samples, weights replicated.

Device-side formulation:
  - all matmuls in bf16 (fp32 PSUM accumulate); weight rows permuted so
    PSUM m-tile (4k+j) holds gate j of NC-chunk k, j order [i,f,g,o]
  - i/f/o weight rows pre-scaled by 0.5 and state stored as 2h/2c so the
    whole cell update needs only tanh():  sigmoid(x) = (tanh(x/2)+1)/2
  - per 16-step group, x-projection (W_x, + bias via a ones-row) and
    lookback term (W_v @ h_{t-32}) are batched matmuls (N=128)
    pre-accumulated into that group's PSUM gates buffer, interleaved as
    background PE work during the previous group's steps
  - per step, i/f/g gate tiles (48 matmuls) run first; the o-gate tiles
    (16 matmuls) + background matmuls execute while the Act/DVE c-path
    (tanh -> c update -> tanh) runs, keeping the PE busy
  - output is the bf16 ring (2h) DMA'd per half-group; host scales 0.5
"""

import numpy as np
import ml_dtypes

import concourse.bass as bass
import concourse.bacc as bacc
import concourse.tile as tile
from concourse import mybir
from concourse.bass import ds
from concourse.bass_utils import run_bass_kernel_spmd

BF16 = mybir.dt.bfloat16
F32 = mybir.dt.float32
AF = mybir.ActivationFunctionType
OP = mybir.AluOpType

NCORES = 8
B, C, H, W = 64, 3, 256, 256
P = 8
NC = 512
F = C * P * P          # 192
G4 = 4 * NC            # 2048
SY = SX = 32
T = SY * SX            # 1024
B1 = B // NCORES       # 8 batch per core
MT = 16                # gate m-tiles of 128
KC = 4                 # NC contraction chunks of 128
SPG = 16               # steps per group (PSUM buffer granularity)
SPB = 32               # steps per loop body (= ring period)
NBODY = T // SPB       # 32

_COMPILED: dict = {}
BGSPLIT = 2            # background matmuls issued in SPG/2-step halves
PEPAD = 0


def _build(nbody: int, repeats: int):
    nc = bacc.Bacc("TRN2", target_bir_lowering=False, debug=False,
                   num_devices=NCORES)
    t_total = nbody * SPB
    xq_d = nc.dram_tensor("xq", [128, 2, t_total + SPB, B1], BF16,
                          kind="ExternalInput").ap()
    whh_d = nc.dram_tensor("whhT", [128, KC * G4], BF16,
                           kind="ExternalInput").ap()
    wv_d = nc.dram_tensor("wvT", [128, KC * G4], BF16,
                          kind="ExternalInput").ap()
    wx_d = nc.dram_tensor("wxT", [128, 2 * G4], BF16,
                          kind="ExternalInput").ap()
    ho_d = nc.dram_tensor("ho", [128, t_total, KC, B1], BF16,
                          kind="ExternalOutput").ap()

    with tile.TileContext(nc) as tc:
        with (
            tc.tile_pool(name="persist", bufs=1) as pp,
            tc.tile_pool(name="ew", bufs=3) as ew,
            tc.tile_pool(name="psum", bufs=1, space="PSUM") as psp,
        ):
            w_hh = pp.tile([128, KC * G4], BF16, tag="w_hh")
            w_v = pp.tile([128, KC * G4], BF16, tag="w_v")
            w_x = pp.tile([128, 2 * G4], BF16, tag="w_x")
            ring = pp.tile([128, SPB, KC, B1], BF16, tag="ring")
            c2 = [pp.tile([128, KC, B1], F32, tag=f"c2_{i}", name=f"c2_{i}") for i in (0, 1)]
            xq_t = [pp.tile([128, 2, SPG, B1], BF16, tag=f"xq_{i}", name=f"xq_{i}")
                    for i in (0, 1)]  # [even-group, odd-group]
            # per group-parity, separate ifg / o tiles: the o matmuls of
            # step t must not address-overlap the ifg act's read (Tile
            # tracks deps by address range), or every step pays a full
            # PE<->Act semaphore rendezvous
            gifg = [psp.tile([128, 12, SPG, B1], F32, tag=f"gi_{i}",
                             name=f"gi_{i}") for i in (0, 1)]
            go = [psp.tile([128, KC, SPG, B1], F32, tag=f"go_{i}",
                           name=f"go_{i}") for i in (0, 1)]

            nc.sync.dma_start(w_hh[:], whh_d)
            nc.sync.dma_start(w_v[:], wv_d)
            nc.sync.dma_start(w_x[:], wx_d)
            nc.vector.memset(ring[:], 0.0)
            nc.vector.memset(c2[0][:], 0.0)
            nc.vector.memset(c2[1][:], 0.0)

            def out_slice(gl, m):
                """(tile, row) for gate m-tile m = 4k+j: j<3 -> ifg tile
                row 3k+j, j==3 -> o tile row k."""
                k, j = m // 4, m % 4
                if j < 3:
                    return gifg[gl], 3 * k + j
                return go[gl], k

            def emit_bg(gl, xq, s0):
                """Background matmuls pre-accumulating one group's gates:
                W_v @ h2[t-32] (ring slots s0..s0+15, no DMA dependency,
                emitted first; start=True clears a whole PSUM bank on HW so
                it is set only on the first matmul into each bank) then
                x-projection (+bias row) from the xq tile."""
                ops = []
                ss = SPG // BGSPLIT
                for m in range(MT):
                    for k in range(KC):
                        for hh in range(BGSPLIT):
                            def op(m=m, k=k, hh=hh):
                                gt, r = out_slice(gl, m)
                                nc.tensor.matmul(
                                    gt[:, r, hh * ss:(hh + 1) * ss, :],
                                    w_v[:, k * G4 + m * 128:k * G4 + (m + 1) * 128],
                                    ring[:, s0 + hh * ss:s0 + (hh + 1) * ss, k, :],
                                    start=(k == 0 and r % 4 == 0 and hh == 0),
                                    stop=False, skip_group_check=True)
                            ops.append(op)
                for m in range(MT):
                    for kc in range(2):
                        for hh in range(BGSPLIT):
                            def op(m=m, kc=kc, hh=hh):
                                gt, r = out_slice(gl, m)
                                nc.tensor.matmul(
                                    gt[:, r, hh * ss:(hh + 1) * ss, :],
                                    w_x[:, kc * G4 + m * 128:kc * G4 + (m + 1) * 128],
                                    xq[:, kc, hh * ss:(hh + 1) * ss, :],
                                    start=False, stop=False,
                                    skip_group_check=True)
                            ops.append(op)
                return ops

            def emit_step(lt, bg_ops):
                """One LSTM step lt (0..31) within the body.

                m-tile 4k+j holds gate j (order [i,f,g,o]) of cell-chunk k.
                i/f/g tiles (j<3) run first; o tiles + bg ops fill the PE
                while the Act/DVE c-path runs.
                """
                gl, lt_g = lt // SPG, lt % SPG
                gi, gto = gifg[gl], go[gl]
                prev = (lt - 1) % SPB
                last_step = lt_g == SPG - 1
                for j in range(3):
                    for k in range(KC):
                        for kc in range(KC):
                            nc.tensor.matmul(
                                gi[:, 3 * k + j, lt_g, :],
                                w_hh[:, kc * G4 + (4 * k + j) * 128:
                                     kc * G4 + (4 * k + j + 1) * 128],
                                ring[:, prev, kc, :],
                                start=False,
                                stop=(last_step and kc == KC - 1),
                                skip_group_check=True)
                # tanh over i/f/g rows; gi viewed as [p, k, j, lt, b]
                g5 = gi[:].rearrange("p (k j) t b -> p k j t b", j=3)
                t4 = ew.tile([128, KC, 4, B1], F32, tag="t4", name="t4")
                nc.scalar.activation(t4[:, :, 0:3, :], g5[:, :, :, lt_g, :],
                                     AF.Tanh)
                for k in range(KC):
                    for kc in range(KC):
                        nc.tensor.matmul(
                            gto[:, k, lt_g, :],
                            w_hh[:, kc * G4 + (4 * k + 3) * 128:
                                 kc * G4 + (4 * k + 4) * 128],
                            ring[:, prev, kc, :],
                            start=False,
                            stop=(last_step and kc == KC - 1),
                            skip_group_check=True)
                for op in bg_ops:
                    op()
                for _pd in range(PEPAD):
                    nc.tensor.ldweights(w_hh[:, 0:128])
                nc.scalar.activation(t4[:, :, 3, :], gto[:, :, lt_g, :],
                                     AF.Tanh)
                b2 = ew.tile([128, KC, B1], F32, tag="b2", name="b2")
                a2 = ew.tile([128, KC, B1], F32, tag="a2", name="a2")
                tch = ew.tile([128, KC, B1], F32, tag="tch", name="tch")
                # b2 = (t_i+1)*t_g = 2 si * tanh(g)
                nc.vector.scalar_tensor_tensor(
                    b2[:], t4[:, :, 0, :], 1.0, t4[:, :, 2, :], OP.add, OP.mult)
                # a2 = (t_f+1)*c2_old = 4 sf * c_old
                nc.vector.scalar_tensor_tensor(
                    a2[:], t4[:, :, 1, :], 1.0, c2[1 - lt % 2][:], OP.add, OP.mult)
                # c2_new = a2/2 + b2 = 2 c_new
                nc.vector.scalar_tensor_tensor(
                    c2[lt % 2][:], a2[:], 0.5, b2[:], OP.mult, OP.add)
                # tch = tanh(c_new)
                nc.scalar.activation(tch[:], c2[lt % 2][:], AF.Tanh, scale=0.5)
                # h2 = (t_o+1)*tch = 2 h  -> ring (bf16, feeds matmuls and
                # is DMA'd out as the output)
                nc.vector.scalar_tensor_tensor(
                    ring[:, lt, :, :], t4[:, :, 3, :], 1.0, tch[:],
                    OP.add, OP.mult)

            # prologue: group 0 inputs + gates
            nc.sync.dma_start(xq_t[0][:], xq_d[:, :, 0:SPG, :])
            for op in emit_bg(0, xq_t[0], 0):
                op()

            def body(i):
                # xq for group 2i+1 (used by bg during gl=0)
                nc.sync.dma_start(xq_t[1][:],
                                  xq_d[:, :, ds(i * SPB + SPG, SPG), :])
                bg = emit_bg(1, xq_t[1], SPG)  # group 2i+1
                nper = (len(bg) + SPG - 1) // SPG
                for lt in range(SPG):
                    emit_step(lt, bg[lt * nper:(lt + 1) * nper])
                # group 0-15 output: ring slots 0..15 final for steps
                # 32i..32i+15; a full group of slack before overwrite
                nc.sync.dma_start(ho_d[:, ds(i * SPB, SPG), :, :],
                                  ring[:, 0:SPG, :, :])
                # xq for group 2i+2 (used by bg during gl=1)
                nc.sync.dma_start(xq_t[0][:],
                                  xq_d[:, :, ds(i * SPB + SPB, SPG), :])
                bg = emit_bg(0, xq_t[0], 0)    # group 2i+2
                for lt in range(SPG, SPB):
                    emit_step(lt, bg[(lt - SPG) * nper:(lt - SPG + 1) * nper])
                nc.sync.dma_start(ho_d[:, ds(i * SPB + SPG, SPG), :, :],
                                  ring[:, SPG:SPB, :, :])

            if repeats == 1:
                with tc.For_i(0, nbody, 1,
                              hint_engines=(mybir.EngineType.PE,)) as i:
                    body(i)
            else:
                with tc.For_i(0, repeats, 1) as _r:
                    with tc.For_i(0, nbody, 1,
                                  hint_engines=(mybir.EngineType.PE,)) as i:
                        body(i)

    nc.compile()
    return nc


def _get(nbody: int, repeats: int):
    key = (nbody, repeats)
    if key not in _COMPILED:
        _COMPILED[key] = _build(nbody, repeats)
    return _COMPILED[key]


def _perm_idx():
    """Permuted gate-row order: m-tile (4k+j) = gate j of NC-chunk k,
    j order [i,f,g,o]; torch gate blocks i=0,f=1,g=2,o=3."""
    gid = [0, 1, 2, 3]
    scl = [0.5, 0.5, 1.0, 0.5]
    idx = np.empty(G4, np.int64)
    rs = np.empty(G4, np.float32)
    for k in range(KC):
        for j in range(4):
            m = 4 * k + j
            idx[m * 128:(m + 1) * 128] = 512 * gid[j] + 128 * k + np.arange(128)
            rs[m * 128:(m + 1) * 128] = scl[j]
    return idx, rs


def _lhsT_pack(wp: np.ndarray) -> np.ndarray:
    """[G4, 512] permuted+scaled weight -> [128, 4*G4] bf16 lhsT tiles:
    out[p, k*G4 + m*128 + c] = wp[m*128+c, 128k+p]."""
    a = wp.reshape(MT, 128, KC, 128).transpose(3, 2, 0, 1).reshape(128, KC * G4)
    return np.ascontiguousarray(a.astype(ml_dtypes.bfloat16))


def _prep_weights(W_ih, W_hh, b_ih, b_hh):
    idx, rs = _perm_idx()
    bias = (np.asarray(b_ih, np.float32) + np.asarray(b_hh, np.float32))[idx] * rs
    Wih_p = np.asarray(W_ih, np.float32)[idx] * rs[:, None]
    Whh_p = np.asarray(W_hh, np.float32)[idx] * rs[:, None] * 0.5
    Wv_p = Wih_p[:, F:] * 0.5
    Wx_p = Wih_p[:, :F]
    whhT = _lhsT_pack(Whh_p)
    wvT = _lhsT_pack(Wv_p)
    wxT = np.zeros((128, 2 * G4), np.float32)
    # chunk 0: features 0..127 ; chunk 1: features 128..191 + bias row 64
    wxT[:, :G4] = Wx_p.reshape(MT, 128, F)[:, :, :128].transpose(2, 0, 1).reshape(128, G4)
    wxT[:64, G4:] = Wx_p.reshape(MT, 128, F)[:, :, 128:].transpose(2, 0, 1).reshape(64, G4)
    wxT[64, G4:] = bias
    return whhT, wvT, np.ascontiguousarray(wxT.astype(ml_dtypes.bfloat16))


def _prep_xq(x_core: np.ndarray, t_total: int) -> np.ndarray:
    """x_core (B1,C,H,W) -> [128, 2, t_total+SPB, B1] bf16 with ones row."""
    xp = (x_core.reshape(B1, C, SY, P, SX, P)
          .transpose(2, 4, 0, 1, 3, 5).reshape(T, B1, F))
    xpT = xp.transpose(2, 0, 1)  # [F, T, B1]
    xq = np.zeros((128, 2, t_total + SPB, B1), np.float32)
    tt = min(T, t_total)
    xq[:, 0, :tt, :] = xpT[:128, :tt]
    xq[:64, 1, :tt, :] = xpT[128:, :tt]
    xq[64, 1, :tt, :] = 1.0
    return np.ascontiguousarray(xq.astype(ml_dtypes.bfloat16))


def _in_maps(x, W_ih, W_hh, b_ih, b_hh, t_total=T):
    whhT, wvT, wxT = _prep_weights(W_ih, W_hh, b_ih, b_hh)
    x = np.asarray(x, np.float32)
    maps = []
    for j in range(NCORES):
        maps.append({
            "xq": _prep_xq(x[j * B1:(j + 1) * B1], t_total),
            "whhT": whhT, "wvT": wvT, "wxT": wxT,
        })
    return maps


def _assemble(results, t_total=T):
    """results[j]["ho"] [128, t_total, KC, B1] bf16 (= 2h) -> (B, NC, SY, SX).

    Matches the reference's to_image exactly: (B, T, NC) row-major data
    reinterpreted as (B, NC, sy, sx) -- T and NC deliberately interleave."""
    out = np.empty((B, t_total, NC), np.float32)
    for j in range(NCORES):
        ho = results[j]["ho"].astype(np.float32)  # [128(p), t, 4(k), 8(b)]
        out[j * B1:(j + 1) * B1] = 0.5 * ho.transpose(3, 1, 2, 0).reshape(
            B1, t_total, NC)
    return out.reshape(B, NC, t_total // SX, SX)


def kernel(x, W_ih, W_hh, b_ih, b_hh):
    nc = _get(NBODY, 1)
    maps = _in_maps(x, W_ih, W_hh, b_ih, b_hh)
    res = run_bass_kernel_spmd(nc, maps, core_ids=list(range(NCORES)))
    return _assemble(res.results)


# revision 10
# speedup vs baseline: 66.1907x; 1.0142x over previous
"""Trainium2 Bass kernel for the 2D-patch LSTM (nn_Lstm2D).

Math (reference): row-major scan over 32x32 grid of 8x8 patches of a
(64,3,256,256) image. Per step t: gates = [x_t, h_{t-32}] @ W_ih.T +
h_{t-1} @ W_hh.T + b; standard LSTM cell update (i,f,g,o torch order).
Strictly sequential chain of T=1024 steps (h,c carry across row
boundaries), so the kernel runs the full scan per core and shards the
batch: 8 cores x 8 samples, weights replicated.

Device-side formulation:
  - all matmuls in bf16 (fp32 PSUM accumulate); weight rows permuted so
    PSUM m-tile (4k+j) holds gate j of NC-chunk k, j order [i,f,g,o]
  - i/f/o weight rows pre-scaled by 0.5 and state stored as 2h/2c so the
    whole cell update needs only tanh():  sigmoid(x) = (tanh(x/2)+1)/2
  - per 16-step group, x-projection (W_x, + bias via a ones-row) and
    lookback term (W_v @ h_{t-32}) are batched matmuls (N=128)
    pre-accumulated into that group's PSUM gates buffer, interleaved as
    background PE work during the previous group's steps
  - per step, i/f/g gate tiles (48 matmuls) run first; the o-gate tiles
    (16 matmuls) + background matmuls execute while the Act/DVE c-path
    (tanh -> c update -> tanh) runs, keeping the PE busy
  - output is the bf16 ring (2h) DMA'd per half-group; host scales 0.5
"""

import numpy as np
import ml_dtypes

import concourse.bass as bass
import concourse.bacc as bacc
import concourse.tile as tile
from concourse import mybir
from concourse.bass import ds
from concourse.bass_utils import run_bass_kernel_spmd

BF16 = mybir.dt.bfloat16
F32 = mybir.dt.float32
AF = mybir.ActivationFunctionType
OP = mybir.AluOpType

NCORES = 8
B, C, H, W = 64, 3, 256, 256
P = 8
NC = 512
F = C * P * P          # 192
G4 = 4 * NC            # 2048
SY = SX = 32
T = SY * SX            # 1024
B1 = B // NCORES       # 8 batch per core
MT = 16                # gate m-tiles of 128
KC = 4                 # NC contraction chunks of 128
SPG = 16               # steps per group (PSUM buffer granularity)
SPB = 32               # steps per loop body (= ring period)
NBODY = T // SPB       # 32

_COMPILED: dict = {}
BGSPLIT = 2            # background matmuls issued in SPG/2-step halves
PEPAD = 0


def _build(nbody: int, repeats: int):
    nc = bacc.Bacc("TRN2", target_bir_lowering=False, debug=False,
                   num_devices=NCORES)
    t_total = nbody * SPB
    xq_d = nc.dram_tensor("xq", [128, 2, t_total + SPB, B1], BF16,
                          kind="ExternalInput").ap()
    whh_d = nc.dram_tensor("whhT", [128, KC * G4], BF16,
                           kind="ExternalInput").ap()
    wv_d = nc.dram_tensor("wvT", [128, KC * G4], BF16,
                          kind="ExternalInput").ap()
    wx_d = nc.dram_tensor("wxT", [128, 2 * G4], BF16,
                          kind="ExternalInput").ap()
    ho_d = nc.dram_tensor("ho", [128, t_total, KC, B1], BF16,
                          kind="ExternalOutput").ap()

    with tile.TileContext(nc) as tc:
        with (
            tc.tile_pool(name="persist", bufs=1) as pp,
            tc.tile_pool(name="ew", bufs=3) as ew,
            tc.tile_pool(name="psum", bufs=1, space="PSUM") as psp,
        ):
            w_hh = pp.tile([128, KC * G4], BF16, tag="w_hh")
            w_v = pp.tile([128, KC * G4], BF16, tag="w_v")
            w_x = pp.tile([128, 2 * G4], BF16, tag="w_x")
            ring = pp.tile([128, SPB, KC, B1], BF16, tag="ring")
            c2 = [pp.tile([128, KC, B1], F32, tag=f"c2_{i}", name=f"c2_{i}") for i in (0, 1)]
            xq_t = [pp.tile([128, 2, SPG, B1], BF16, tag=f"xq_{i}", name=f"xq_{i}")
                    for i in (0, 1)]  # [even-group, odd-group]
            # per group-parity, separate ifg / o tiles: the o matmuls of
            # step t must not address-overlap the ifg act's read (Tile
            # tracks deps by address range), or every step pays a full
            # PE<->Act semaphore rendezvous
            gifg = [psp.tile([128, 12, SPG, B1], F32, tag=f"gi_{i}",
                             name=f"gi_{i}") for i in (0, 1)]
            go = [psp.tile([128, KC, SPG, B1], F32, tag=f"go_{i}",
                           name=f"go_{i}") for i in (0, 1)]

            nc.sync.dma_start(w_hh[:], whh_d)
            nc.sync.dma_start(w_v[:], wv_d)
            nc.sync.dma_start(w_x[:], wx_d)
            nc.vector.memset(ring[:], 0.0)
            nc.vector.memset(c2[0][:], 0.0)
            nc.vector.memset(c2[1][:], 0.0)

            def out_slice(gl, m):
                """(tile, row) for gate m-tile m = 4k+j: j<3 -> ifg tile
                row 3k+j, j==3 -> o tile row k."""
                k, j = m // 4, m % 4
                if j < 3:
                    return gifg[gl], 3 * k + j
                return go[gl], k

            def emit_bg(gl, xq, s0):
                """Background matmuls pre-accumulating one group's gates:
                W_v @ h2[t-32] (ring slots s0..s0+15, no DMA dependency,
                emitted first; start=True clears a whole PSUM bank on HW so
                it is set only on the first matmul into each bank) then
                x-projection (+bias row) from the xq tile."""
                ops = []
                ss = SPG // BGSPLIT
                for m in range(MT):
                    for k in range(KC):
                        for hh in range(BGSPLIT):
                            def op(m=m, k=k, hh=hh):
                                gt, r = out_slice(gl, m)
                                nc.tensor.matmul(
                                    gt[:, r, hh * ss:(hh + 1) * ss, :],
                                    w_v[:, k * G4 + m * 128:k * G4 + (m + 1) * 128],
                                    ring[:, s0 + hh * ss:s0 + (hh + 1) * ss, k, :],
                                    start=(k == 0 and r % 4 == 0 and hh == 0),
                                    stop=False, skip_group_check=True)
                            ops.append(op)
                for m in range(MT):
                    for kc in range(2):
                        for hh in range(BGSPLIT):
                            def op(m=m, kc=kc, hh=hh):
                                gt, r = out_slice(gl, m)
                                nc.tensor.matmul(
                                    gt[:, r, hh * ss:(hh + 1) * ss, :],
                                    w_x[:, kc * G4 + m * 128:kc * G4 + (m + 1) * 128],
                                    xq[:, kc, hh * ss:(hh + 1) * ss, :],
                                    start=False, stop=False,
                                    skip_group_check=True)
                            ops.append(op)
                return ops

            def emit_step(lt, bg_ops):
                """One LSTM step lt (0..31) within the body.

                m-tile 4k+j holds gate j (order [i,f,g,o]) of cell-chunk k.
                i/f/g tiles (j<3) run first; o tiles + bg ops fill the PE
                while the Act/DVE c-path runs.
                """
                gl, lt_g = lt // SPG, lt % SPG
                gi, gto = gifg[gl], go[gl]
                prev = (lt - 1) % SPB
                last_step = lt_g == SPG - 1
                # kc-outer: consecutive matmuls never accumulate onto the
                # same PSUM address back-to-back, and the first 24 matmuls
                # of a step need only ring chunks 0/1 (written first by the
                # previous step's split ring update)
                for kc in range(KC):
                    for j in range(3):
                        for k in range(KC):
                            nc.tensor.matmul(
                                gi[:, 3 * k + j, lt_g, :],
                                w_hh[:, kc * G4 + (4 * k + j) * 128:
                                     kc * G4 + (4 * k + j + 1) * 128],
                                ring[:, prev, kc, :],
                                start=False,
                                stop=(last_step and kc == KC - 1),
                                skip_group_check=True)
                # tanh over i/f/g rows; gi viewed as [p, k, j, lt, b]
                g5 = gi[:].rearrange("p (k j) t b -> p k j t b", j=3)
                t4 = ew.tile([128, KC, 4, B1], F32, tag="t4", name="t4")
                nc.scalar.activation(t4[:, :, 0:3, :], g5[:, :, :, lt_g, :],
                                     AF.Tanh)
                for kc in range(KC):
                    for k in range(KC):
                        nc.tensor.matmul(
                            gto[:, k, lt_g, :],
                            w_hh[:, kc * G4 + (4 * k + 3) * 128:
                                 kc * G4 + (4 * k + 4) * 128],
                            ring[:, prev, kc, :],
                            start=False,
                            stop=(last_step and kc == KC - 1),
                            skip_group_check=True)
                for op in bg_ops:
                    op()
                for _pd in range(PEPAD):
                    nc.tensor.ldweights(w_hh[:, 0:128])
                nc.scalar.activation(t4[:, :, 3, :], gto[:, :, lt_g, :],
                                     AF.Tanh)
                b2 = ew.tile([128, KC, B1], F32, tag="b2", name="b2")
                a2 = ew.tile([128, KC, B1], F32, tag="a2", name="a2")
                tch = ew.tile([128, KC, B1], F32, tag="tch", name="tch")
                # b2 = (t_i+1)*t_g = 2 si * tanh(g)
                nc.vector.scalar_tensor_tensor(
                    b2[:], t4[:, :, 0, :], 1.0, t4[:, :, 2, :], OP.add, OP.mult)
                # a2 = (t_f+1)*c2_old = 4 sf * c_old
                nc.vector.scalar_tensor_tensor(
                    a2[:], t4[:, :, 1, :], 1.0, c2[1 - lt % 2][:], OP.add, OP.mult)
                # c2_new = a2/2 + b2 = 2 c_new
                nc.vector.scalar_tensor_tensor(
                    c2[lt % 2][:], a2[:], 0.5, b2[:], OP.mult, OP.add)
                # tch = tanh(c_new)
                nc.scalar.activation(tch[:], c2[lt % 2][:], AF.Tanh, scale=0.5)
                # h2 = (t_o+1)*tch = 2 h  -> ring (bf16, feeds matmuls and
                # is DMA'd out as the output). Written in two k-halves so
                # the next step's kc<2 matmuls can start on the first half.
                nc.vector.scalar_tensor_tensor(
                    ring[:, lt, 0:2, :], t4[:, 0:2, 3, :], 1.0, tch[:, 0:2, :],
                    OP.add, OP.mult)
                nc.vector.scalar_tensor_tensor(
                    ring[:, lt, 2:4, :], t4[:, 2:4, 3, :], 1.0, tch[:, 2:4, :],
                    OP.add, OP.mult)

            # prologue: group 0 inputs + gates
            nc.sync.dma_start(xq_t[0][:], xq_d[:, :, 0:SPG, :])
            for op in emit_bg(0, xq_t[0], 0):
                op()

            def body(i):
                # xq for group 2i+1 (used by bg during gl=0)
                nc.sync.dma_start(xq_t[1][:],
                                  xq_d[:, :, ds(i * SPB + SPG, SPG), :])
                bg = emit_bg(1, xq_t[1], SPG)  # group 2i+1
                nper = (len(bg) + SPG - 1) // SPG
                for lt in range(SPG):
                    emit_step(lt, bg[lt * nper:(lt + 1) * nper])
                # group 0-15 output: ring slots 0..15 final for steps
                # 32i..32i+15; a full group of slack before overwrite
                nc.sync.dma_start(ho_d[:, ds(i * SPB, SPG), :, :],
                                  ring[:, 0:SPG, :, :])
                # xq for group 2i+2 (used by bg during gl=1)
                nc.sync.dma_start(xq_t[0][:],
                                  xq_d[:, :, ds(i * SPB + SPB, SPG), :])
                bg = emit_bg(0, xq_t[0], 0)    # group 2i+2
                for lt in range(SPG, SPB):
                    emit_step(lt, bg[(lt - SPG) * nper:(lt - SPG + 1) * nper])
                nc.sync.dma_start(ho_d[:, ds(i * SPB + SPG, SPG), :, :],
                                  ring[:, SPG:SPB, :, :])

            if repeats == 1:
                with tc.For_i(0, nbody, 1,
                              hint_engines=(mybir.EngineType.PE,)) as i:
                    body(i)
            else:
                with tc.For_i(0, repeats, 1) as _r:
                    with tc.For_i(0, nbody, 1,
                                  hint_engines=(mybir.EngineType.PE,)) as i:
                        body(i)

    nc.compile()
    return nc


def _get(nbody: int, repeats: int):
    key = (nbody, repeats)
    if key not in _COMPILED:
        _COMPILED[key] = _build(nbody, repeats)
    return _COMPILED[key]


def _perm_idx():
    """Permuted gate-row order: m-tile (4k+j) = gate j of NC-chunk k,
    j order [i,f,g,o]; torch gate blocks i=0,f=1,g=2,o=3."""
    gid = [0, 1, 2, 3]
    scl = [0.5, 0.5, 1.0, 0.5]
    idx = np.empty(G4, np.int64)
    rs = np.empty(G4, np.float32)
    for k in range(KC):
        for j in range(4):
            m = 4 * k + j
            idx[m * 128:(m + 1) * 128] = 512 * gid[j] + 128 * k + np.arange(128)
            rs[m * 128:(m + 1) * 128] = scl[j]
    return idx, rs


def _lhsT_pack(wp: np.ndarray) -> np.ndarray:
    """[G4, 512] permuted+scaled weight -> [128, 4*G4] bf16 lhsT tiles:
    out[p, k*G4 + m*128 + c] = wp[m*128+c, 128k+p]."""
    a = wp.reshape(MT, 128, KC, 128).transpose(3, 2, 0, 1).reshape(128, KC * G4)
    return np.ascontiguousarray(a.astype(ml_dtypes.bfloat16))


def _prep_weights(W_ih, W_hh, b_ih, b_hh):
    idx, rs = _perm_idx()
    bias = (np.asarray(b_ih, np.float32) + np.asarray(b_hh, np.float32))[idx] * rs
    Wih_p = np.asarray(W_ih, np.float32)[idx] * rs[:, None]
    Whh_p = np.asarray(W_hh, np.float32)[idx] * rs[:, None] * 0.5
    Wv_p = Wih_p[:, F:] * 0.5
    Wx_p = Wih_p[:, :F]
    whhT = _lhsT_pack(Whh_p)
    wvT = _lhsT_pack(Wv_p)
    wxT = np.zeros((128, 2 * G4), np.float32)
    # chunk 0: features 0..127 ; chunk 1: features 128..191 + bias row 64
    wxT[:, :G4] = Wx_p.reshape(MT, 128, F)[:, :, :128].transpose(2, 0, 1).reshape(128, G4)
    wxT[:64, G4:] = Wx_p.reshape(MT, 128, F)[:, :, 128:].transpose(2, 0, 1).reshape(64, G4)
    wxT[64, G4:] = bias
    return whhT, wvT, np.ascontiguousarray(wxT.astype(ml_dtypes.bfloat16))


def _prep_xq(x_core: np.ndarray, t_total: int) -> np.ndarray:
    """x_core (B1,C,H,W) -> [128, 2, t_total+SPB, B1] bf16 with ones row."""
    xp = (x_core.reshape(B1, C, SY, P, SX, P)
          .transpose(2, 4, 0, 1, 3, 5).reshape(T, B1, F))
    xpT = xp.transpose(2, 0, 1)  # [F, T, B1]
    xq = np.zeros((128, 2, t_total + SPB, B1), np.float32)
    tt = min(T, t_total)
    xq[:, 0, :tt, :] = xpT[:128, :tt]
    xq[:64, 1, :tt, :] = xpT[128:, :tt]
    xq[64, 1, :tt, :] = 1.0
    return np.ascontiguousarray(xq.astype(ml_dtypes.bfloat16))


def _in_maps(x, W_ih, W_hh, b_ih, b_hh, t_total=T):
    whhT, wvT, wxT = _prep_weights(W_ih, W_hh, b_ih, b_hh)
    x = np.asarray(x, np.float32)
    maps = []
    for j in range(NCORES):
        maps.append({
            "xq": _prep_xq(x[j * B1:(j + 1) * B1], t_total),
            "whhT": whhT, "wvT": wvT, "wxT": wxT,
        })
    return maps


def _assemble(results, t_total=T):
    """results[j]["ho"] [128, t_total, KC, B1] bf16 (= 2h) -> (B, NC, SY, SX).

    Matches the reference's to_image exactly: (B, T, NC) row-major data
    reinterpreted as (B, NC, sy, sx) -- T and NC deliberately interleave."""
    out = np.empty((B, t_total, NC), np.float32)
    for j in range(NCORES):
        ho = results[j]["ho"].astype(np.float32)  # [128(p), t, 4(k), 8(b)]
        out[j * B1:(j + 1) * B1] = 0.5 * ho.transpose(3, 1, 2, 0).reshape(
            B1, t_total, NC)
    return out.reshape(B, NC, t_total // SX, SX)


def kernel(x, W_ih, W_hh, b_ih, b_hh):
    nc = _get(NBODY, 1)
    maps = _in_maps(x, W_ih, W_hh, b_ih, b_hh)
    res = run_bass_kernel_spmd(nc, maps, core_ids=list(range(NCORES)))
    return _assemble(res.results)
